# revision 5
# baseline (speedup 1.0000x reference)
"""AttentionBlock (GroupNorm + single-head attention + residual) on 8 TRN2 NeuronCores.

Sharding: data-parallel over batch B=16 -> 2 items per core; weights replicated;
no collectives. All heavy matmuls run in float32r (full PE rate, ~1e-4 rel err).

Per-core pipeline per batch item:
  GroupNorm stats via free-dim reduces + tiny cross-partition matmuls ->
  affine normalize on ScalarE -> q/k (c-major) and vT (n-major) projections ->
  S = qT k -> exp on ScalarE (no max-subtraction; inputs are unit-scale) with
  fused row-sum -> row-normalize on VectorE -> PE-transpose P -> out = vT.T @ pT
  (+bv folded, softmax rows sum to 1) -> final projection + bp + residual.

Engine choreography keeps every instruction to at most ONE semaphore wait
(walrus rejects more): all matmul/ACT inputs are produced by a single engine,
params are funneled through VectorE rounding copies at start.
"""
import numpy as np
from contextlib import ExitStack

B, C, HW = 16, 512, 1024
NCORES = 8
BPC = B // NCORES  # batch items per core
NG = 8             # groupnorm groups
EPS = 1e-5
GSIZE = (C // NG) * HW  # elements per group = 64 * 1024

_CACHE = {}


def _build():
    from concourse import bacc, mybir, tile

    F32 = mybir.dt.float32
    F32R = mybir.dt.float32r
    AF = mybir.ActivationFunctionType
    ALU = mybir.AluOpType

    nc = bacc.Bacc("TRN2", target_bir_lowering=False, debug=False, num_devices=NCORES)

    x_d = nc.dram_tensor("x", (BPC, C, HW), F32, kind="ExternalInput")
    w_d = {n: nc.dram_tensor(n, (C, C), F32, kind="ExternalInput")
           for n in ("wqT", "wkT", "wvT", "wpT")}
    p_d = {n: nc.dram_tensor(n, (C,), F32, kind="ExternalInput")
           for n in ("bq", "bk", "bv", "bp", "gamma", "beta")}
    ident_d = nc.dram_tensor("ident", (128, 128), F32, kind="ExternalInput")
    e64_d = nc.dram_tensor("e64", (128, 2), F32, kind="ExternalInput")
    hsel_d = nc.dram_tensor("hsel", (2, 128), F32, kind="ExternalInput")
    y_d = nc.dram_tensor("y", (BPC, C, HW), F32, kind="ExternalOutput")

    with ExitStack() as ctx:
        tc = ctx.enter_context(tile.TileContext(nc))
        wpool = ctx.enter_context(tc.tile_pool(name="wpool", bufs=1))
        cpool = ctx.enter_context(tc.tile_pool(name="cpool", bufs=1))
        xpool = ctx.enter_context(tc.tile_pool(name="xpool", bufs=2))
        apool = ctx.enter_context(tc.tile_pool(name="apool", bufs=1))
        ppool = ctx.enter_context(tc.tile_pool(name="ppool", bufs=2))
        tpool = ctx.enter_context(tc.tile_pool(name="tpool", bufs=1))
        spool = ctx.enter_context(tc.tile_pool(name="spool", bufs=1))
        mm_ps = ctx.enter_context(tc.tile_pool(name="mm_ps", bufs=3, space="PSUM"))
        tr_ps = ctx.enter_context(tc.tile_pool(name="tr_ps", bufs=2, space="PSUM"))
        st_ps = ctx.enter_context(tc.tile_pool(name="st_ps", bufs=1, space="PSUM"))

        # ---- params: DMA raw f32 bytes into f32r-typed tiles, round in place on DVE
        wsb = {}
        for n in ("wqT", "wkT", "wvT", "wpT"):
            wstage = wpool.tile([128, 4, 512], F32, name="wstage", tag="wstage")
            nc.sync.dma_start(wstage[:], w_d[n].rearrange("(t p) m -> p t m", p=128))
            wt = wpool.tile([128, 4, 512], F32R, name=f"w_{n}")
            nc.vector.tensor_copy(wt[:], wstage[:])
            wsb[n] = wt
        psb = {}
        for n in ("bq", "bk", "bv", "bp", "gamma", "beta"):
            pt = cpool.tile([128, 4], F32, name=f"p_{n}")
            nc.sync.dma_start(pt[:], p_d[n].rearrange("(t p) -> p t", p=128))
            nc.vector.tensor_copy(pt[:], pt[:])
            psb[n] = pt
        istage = cpool.tile([128, 128], F32)
        nc.sync.dma_start(istage[:], ident_d[:])
        ident = cpool.tile([128, 128], F32R)
        nc.vector.tensor_copy(ident[:], istage[:])
        e64 = cpool.tile([128, 2], F32)
        nc.sync.dma_start(e64[:], e64_d[:])
        nc.vector.tensor_copy(e64[:], e64[:])
        hsel = cpool.tile([2, 128], F32)
        nc.sync.dma_start(hsel[:], hsel_d[:])
        nc.vector.tensor_copy(hsel[:], hsel[:])

        scale = 1.0 / (C ** 0.5)
        inv_n = 1.0 / GSIZE

        for it in range(BPC):
            x_sb = xpool.tile([128, 4, HW], F32, name="x_sb")
            nc.sync.dma_start(x_sb[:], x_d[it].rearrange("(t p) n -> p t n", p=128))

            # ---- GroupNorm stats
            ssum = spool.tile([128, 4], F32, name="ssum", bufs=2)
            ssq = spool.tile([128, 4], F32, name="ssq", bufs=2)
            junk = spool.tile([128, HW], mybir.dt.bfloat16, name="junk", bufs=2)
            for t in range(4):
                nc.vector.tensor_reduce(ssum[:, t:t + 1], x_sb[:, t, :],
                                        axis=mybir.AxisListType.X, op=ALU.add)
            for t in range(4):
                nc.scalar.activation(junk[:], x_sb[:, t, :], AF.Square,
                                     accum_out=ssq[:, t:t + 1])
            gsums = st_ps.tile([2, 4], F32, name="gsums")
            gsqs = st_ps.tile([2, 4], F32, name="gsqs")
            nc.tensor.matmul(gsums[:], e64[:], ssum[:], start=True, stop=True)
            nc.tensor.matmul(gsqs[:], e64[:], ssq[:], start=True, stop=True)
            # stat2 = [mean(4) | rstd(4)] on 2 partitions
            stat2 = spool.tile([2, 8], F32, name="stat2", bufs=2)
            tmp_e = spool.tile([2, 4], F32, name="tmp_e", bufs=2)
            tmp_m = spool.tile([2, 4], F32, name="tmp_m", bufs=2)
            tmp_v = spool.tile([2, 4], F32, name="tmp_v", bufs=2)
            tmp_s = spool.tile([2, 4], F32, name="tmp_s", bufs=2)
            nc.vector.tensor_scalar_mul(stat2[:, 0:4], gsums[:], inv_n)
            nc.vector.tensor_scalar_mul(tmp_e[:], gsqs[:], inv_n)
            nc.vector.tensor_tensor(tmp_m[:], stat2[:, 0:4], stat2[:, 0:4], op=ALU.mult)
            # (ex2 + EPS) - mean^2
            nc.vector.scalar_tensor_tensor(tmp_v[:], tmp_e[:], EPS, tmp_m[:],
                                           op0=ALU.add, op1=ALU.subtract)
            nc.scalar.activation(tmp_s[:], tmp_v[:], AF.Sqrt)
            nc.vector.reciprocal(stat2[:, 4:8], tmp_s[:])
            bcast = st_ps.tile([128, 8], F32, name="bcast")
            nc.tensor.matmul(bcast[:], hsel[:], stat2[:], start=True, stop=True)
            a_sc = spool.tile([128, 4], F32, name="a_sc", bufs=2)
            b_sc = spool.tile([128, 4], F32, name="b_sc", bufs=2)
            tmp_ma = spool.tile([128, 4], F32, name="tmp_ma", bufs=2)
            nc.vector.tensor_tensor(a_sc[:], bcast[:, 4:8], psb["gamma"][:], op=ALU.mult)
            nc.vector.tensor_tensor(tmp_ma[:], bcast[:, 0:4], a_sc[:], op=ALU.mult)
            nc.vector.tensor_tensor(b_sc[:], psb["beta"][:], tmp_ma[:], op=ALU.subtract)

            # ---- normalize: xn = x * a + b   (ScalarE, f32r out)
            xn = apool.tile([128, 4, HW], F32R, name="xn")
            for t in range(4):
                nc.scalar.activation(xn[:, t, :], x_sb[:, t, :], AF.Identity,
                                     bias=b_sc[:, t:t + 1], scale=a_sc[:, t:t + 1])

            # ---- vT first (so its ACT-evac tick is covered by later q/k waits)
            vT = apool.tile([128, 8, 512], F32R, name="vT")
            for nt in range(8):
                ps = mm_ps.tile([128, 512], F32, name="ps_mm")
                for k in range(4):
                    nc.tensor.matmul(ps[:], xn[:, k, nt * 128:(nt + 1) * 128],
                                     wsb["wvT"][:, k, :],
                                     start=(k == 0), stop=(k == 3))
                nc.scalar.activation(vT[:, nt, :], ps[:], AF.Copy)

            # ---- q, k projections (c-major), bias on ACT evac
            q_sb = apool.tile([128, 4, HW], F32R, name="q_sb")
            k_sb = apool.tile([128, 4, HW], F32R, name="k_sb")
            for (dst, wname, bname) in ((q_sb, "wqT", "bq"), (k_sb, "wkT", "bk")):
                for m in range(4):
                    for nn in range(2):
                        ps = mm_ps.tile([128, 512], F32, name="ps_mm")
                        for k in range(4):
                            nc.tensor.matmul(
                                ps[:], wsb[wname][:, k, m * 128:(m + 1) * 128],
                                dst_rhs(xn, k, nn),
                                start=(k == 0), stop=(k == 3))
                        nc.scalar.activation(dst[:, m, nn * 512:(nn + 1) * 512],
                                             ps[:], AF.Identity,
                                             bias=psb[bname][:, m:m + 1])

            # ---- attention, i-groups of 512 so AV keeps N=512
            rsum = spool.tile([128, 16], F32, name="rsum", bufs=2)
            rinv = spool.tile([128, 8], F32, name="rinv", bufs=2)
            attn = apool.tile([128, 4, HW], F32R, name="attn")
            for ig in range(2):
                pT = tpool.tile([128, 8, 512], F32R, name="pT")
                for ic4 in range(4):
                    ic = ig * 4 + ic4
                    ps_s = []
                    for jc in range(2):
                        ps = mm_ps.tile([128, 512], F32, name="ps_mm")
                        for k in range(4):
                            nc.tensor.matmul(
                                ps[:], q_sb[:, k, ic * 128:(ic + 1) * 128],
                                k_sb[:, k, jc * 512:(jc + 1) * 512],
                                start=(k == 0), stop=(k == 3))
                        ps_s.append(ps)
                    p_sb = ppool.tile([128, HW], F32R, name="p_sb")
                    for jc in range(2):
                        nc.scalar.activation(p_sb[:, jc * 512:(jc + 1) * 512],
                                             ps_s[jc][:], AF.Exp, scale=scale,
                                             accum_out=rsum[:, 2 * ic + jc:2 * ic + jc + 1])
                    nc.vector.tensor_tensor(rinv[:, ic:ic + 1],
                                            rsum[:, 2 * ic:2 * ic + 1],
                                            rsum[:, 2 * ic + 1:2 * ic + 2], op=ALU.add)
                    nc.vector.reciprocal(rinv[:, ic:ic + 1], rinv[:, ic:ic + 1])
                    nc.vector.tensor_scalar_mul(p_sb[:], p_sb[:], rinv[:, ic:ic + 1])
                    for jb2 in range(2):
                        ps_t = tr_ps.tile([128, 512], F32R, name="ps_tr")
                        for b in range(4):
                            nc.tensor.transpose(
                                ps_t[:, b * 128:(b + 1) * 128],
                                p_sb[:, (jb2 * 4 + b) * 128:(jb2 * 4 + b + 1) * 128],
                                ident[:])
                        nc.vector.tensor_copy(
                            pT[:, jb2 * 4:(jb2 + 1) * 4, ic4 * 128:(ic4 + 1) * 128],
                            ps_t[:].rearrange("p (a b) -> p a b", a=4))
                # out[c, i-group] = sum_j v[c,j] p[i,j]  (+bv via softmax sum=1)
                for mc in range(4):
                    ps = mm_ps.tile([128, 512], F32, name="ps_mm")
                    for jb in range(8):
                        nc.tensor.matmul(ps[:], vT[:, jb, mc * 128:(mc + 1) * 128],
                                         pT[:, jb, :],
                                         start=(jb == 0), stop=(jb == 7))
                    nc.vector.tensor_scalar_add(attn[:, mc, ig * 512:(ig + 1) * 512],
                                                ps[:], psb["bv"][:, mc:mc + 1])

            # ---- final projection + bias + residual
            y_sb = tpool.tile([128, 4, HW], F32, name="y_sb")
            for m in range(4):
                for nn in range(2):
                    ps = mm_ps.tile([128, 512], F32, name="ps_mm")
                    for k in range(4):
                        nc.tensor.matmul(ps[:], wsb["wpT"][:, k, m * 128:(m + 1) * 128],
                                         attn[:, k, nn * 512:(nn + 1) * 512],
                                         start=(k == 0), stop=(k == 3))
                    nc.vector.scalar_tensor_tensor(
                        y_sb[:, m, nn * 512:(nn + 1) * 512], ps[:],
                        psb["bp"][:, m:m + 1], x_sb[:, m, nn * 512:(nn + 1) * 512],
                        op0=ALU.add, op1=ALU.add)
            nc.sync.dma_start(y_d[it].rearrange("(t p) n -> p t n", p=128), y_sb[:])

    nc.compile()
    return nc


def dst_rhs(xn, k, nn):
    return xn[:, k, nn * 512:(nn + 1) * 512]


def _get_nc():
    if "nc" not in _CACHE:
        _CACHE["nc"] = _build()
    return _CACHE["nc"]


def kernel(x, gamma, beta, wq, bq, wk, bk, wv, bv, wp, bp, **_):
    from concourse.bass_utils import run_bass_kernel_spmd

    nc = _get_nc()
    x = np.ascontiguousarray(np.asarray(x, dtype=np.float32).reshape(B, C, HW))
    common = {
        "wqT": np.ascontiguousarray(np.asarray(wq, np.float32).T),
        "wkT": np.ascontiguousarray(np.asarray(wk, np.float32).T),
        "wvT": np.ascontiguousarray(np.asarray(wv, np.float32).T),
        "wpT": np.ascontiguousarray(np.asarray(wp, np.float32).T),
        "bq": np.asarray(bq, np.float32), "bk": np.asarray(bk, np.float32),
        "bv": np.asarray(bv, np.float32), "bp": np.asarray(bp, np.float32),
        "gamma": np.asarray(gamma, np.float32), "beta": np.asarray(beta, np.float32),
        "ident": np.eye(128, dtype=np.float32),
        "e64": np.stack([(np.arange(128) < 64), (np.arange(128) >= 64)],
                        axis=1).astype(np.float32),
        "hsel": np.stack([(np.arange(128) < 64), (np.arange(128) >= 64)],
                         axis=0).astype(np.float32),
    }
    in_maps = [dict(common, x=x[i * BPC:(i + 1) * BPC]) for i in range(NCORES)]
    res = run_bass_kernel_spmd(nc, in_maps, core_ids=list(range(NCORES)))
    y = np.concatenate([res.results[i]["y"] for i in range(NCORES)], axis=0)
    return y.reshape(B, C, 32, 32).astype(np.float32)
